# revision 18
# baseline (speedup 1.0000x reference)
"""Causal multi-head attention (B=4, T=2048, C=1024, H=16, HD=64) on 8 TRN2
NeuronCores.

Sharding: 2D - batch (4) x head-group (2 groups of 8 heads). Each core handles
one batch's tokens for 8 heads (OC = 512 local channels); host sums the two
group partials of y.

v2 layout/precision strategy:
  - Q/K projections run in fp8e4 DoubleRow mode (x and 32*Wq/32*Wk quantized
    to fp8 on host, contraction 256/instr at 0.5 cyc/row). The 32*32=1024
    factor is folded into the exp scale.
  - Q^T/K^T stored as fp8 [128, 2, T] DoubleRow operands (t=1 plane zeroed,
    qt additionally zero on the other head's 64 rows); S^T chunks
    [k=128, q<=512] via fp8 DoubleRow at 0.5 cyc/row.
  - V projection and output projection in bf16 (1 cyc/row).
  - Causal q-range restriction at 128-granularity on S, exp, and ctx
    (diagonal chunks only compute q >= 128*kc).
  - Masking: DVE multiplies with host-provided lower-tri mask tiles (T0/T1)
    on the two diagonal chunks per (head, block) - gpsimd only does the
    denominator partition_broadcast and v-ones memsets.
  - ctx matmuls bf16 with the [V_h | 1] ones-column denominator trick;
    normalize via broadcast + DVE reciprocal + fused mul into bf16 ct.
  - y^T computed in bf16, host converts/sums in f32.
"""

import numpy as np

B, T_FULL, C = 4, 2048, 1024
H, HD = 16, 64
GROUPS = 2
HL = H // GROUPS          # heads per core = 8
OC = HL * HD              # local channels = 512
P = 128                   # partitions
TB = 512                  # token block (moving dim)
SCALE = float(1.0 / np.sqrt(HD))
NCORES = 8
WSCALE = 32.0             # host premultiplies Wq/Wk before fp8 quantization
VSTRIDE = 72              # per-head column stride in v tiles (64 d + 1 one + 7 pad)


def build_program(T=T_FULL):
    from contextlib import ExitStack

    import concourse.bacc as bacc
    import concourse.mybir as mybir
    import concourse.tile as tile

    f32 = mybir.dt.float32
    bf16 = mybir.dt.bfloat16
    fp8 = mybir.dt.float8e4
    EXP = mybir.ActivationFunctionType.Exp
    DR = mybir.MatmulPerfMode.DoubleRow
    ESCALE = SCALE / (WSCALE * WSCALE)

    NTB = T // TB             # 512-token blocks
    NKC = T // P              # 128-token key chunks
    CCH = C // P              # 8 contraction chunks of C
    DCH = C // (2 * P)        # 4 double-contraction chunks (fp8 DoubleRow)
    MCH = OC // P             # 4 output-channel chunks

    nc = bacc.Bacc("TRN2", target_bir_lowering=False, debug=False)
    xq8d = nc.dram_tensor("xq8", [C // 2, 2 * T], fp8, kind="ExternalInput").ap()
    xb16d = nc.dram_tensor("xb16", [C, T], bf16, kind="ExternalInput").ap()
    wq8d = nc.dram_tensor("wq8", [C // 2, 2 * OC], fp8, kind="ExternalInput").ap()
    wk8d = nc.dram_tensor("wk8", [C // 2, 2 * OC], fp8, kind="ExternalInput").ap()
    wv16d = nc.dram_tensor("wv16", [C, OC], bf16, kind="ExternalInput").ap()
    wo16d = nc.dram_tensor("wo16", [OC, C], bf16, kind="ExternalInput").ap()
    trid = nc.dram_tensor("tri", [P, P], bf16, kind="ExternalInput").ap()
    zerod = nc.dram_tensor("zero8", [P, T], fp8, kind="ExternalInput").ap()
    yT = nc.dram_tensor("yT", [C, T], bf16, kind="ExternalOutput").ap()

    with tile.TileContext(nc) as tc, ExitStack() as ctx:
        perm = ctx.enter_context(tc.tile_pool(name="perm", bufs=1))
        # per-head Q^T fp8 [128, 2, T]: rows (h%2)*64..+64 of t=0 hold the
        # head's channels; everything else stays zero (zero-DMA'd once) so the
        # DoubleRow contraction only sees this head's 64 channels.
        qt = [perm.tile([P, 2 * T], fp8, tag=f"qt{h}", name=f"qt{h}")
              for h in range(HL)]
        # per-pair K^T fp8 [128, 2, T]: t=0 holds both heads' channels
        # (rows = oc chunk), t=1 zero.
        kt = [perm.tile([P, 2 * T], fp8, tag=f"kt{m}", name=f"kt{m}")
              for m in range(MCH)]
        # normalized ctx^T bf16 [oc rows, T] per m-chunk
        ct = [perm.tile([P, T], bf16, tag=f"ct{m}", name=f"ct{m}") for m in range(MCH)]
        # V bf16 per key chunk: head stride 72 = [64 vals | 1 | 7 junk]
        v = [perm.tile([P, HL * VSTRIDE], bf16, tag=f"v{t}", name=f"v{t}")
             for t in range(NKC)]
        tri = perm.tile([P, P], bf16, tag="tri", name="tri")
        ONE_BF16 = 0x3F80
        for vt in v:
            vv = vt.rearrange("p (h e) -> p h e", e=VSTRIDE)
            nc.gpsimd.memset(vv[:, :, 64:65].bitcast(mybir.dt.uint16), ONE_BF16)

        # x resident in SBUF for the whole kernel (48KB)
        xq8 = [perm.tile([P, 2 * T], fp8, tag=f"xq8_{d}", name=f"xq8_{d}")
               for d in range(DCH)]
        xb16 = [perm.tile([P, T], bf16, tag=f"xb{c}", name=f"xb{c}")
                for c in range(CCH)]

        with (
            tc.tile_pool(name="wpool", bufs=1) as wp,
            tc.tile_pool(name="ptpool", bufs=16) as ptp,
            tc.tile_pool(name="tmppool", bufs=2) as tmp,
            tc.tile_pool(name="ypool", bufs=2) as yp,
            tc.tile_pool(name="mmps", bufs=2, space="PSUM") as pp,
            tc.tile_pool(name="stps", bufs=2, space="PSUM") as stp,
            tc.tile_pool(name="ctxps", bufs=2, space="PSUM") as cxp,
        ):
            # ---- loads, ordered so block-0's dependency chain lands first:
            # tri; xq8/wq8/wk8 block-0 slices; per-tile zero planes (gate the
            # first casts / S matmuls); xb16/wv block-0; the rest streams in
            # behind while attend(0) runs.
            nc.sync.dma_start(out=tri, in_=trid)
            wq, wk, wv, wo = [], [], [], []
            xq8v = [x_.rearrange("p (t f) -> p t f", t=2) for x_ in xq8]
            xq8dv = xq8d.rearrange("p (t f) -> p t f", t=2)
            for d in range(DCH):
                nc.sync.dma_start(out=xq8v[d][:, :, 0:TB],
                                  in_=xq8dv[d * P:(d + 1) * P, :, 0:TB])
                t_ = wp.tile([P, 2 * OC], fp8, tag=f"wq{d}", name=f"wq{d}")
                nc.sync.dma_start(out=t_, in_=wq8d[d * P:(d + 1) * P, :])
                wq.append(t_)
            for d in range(DCH):
                t_ = wp.tile([P, 2 * OC], fp8, tag=f"wk{d}", name=f"wk{d}")
                nc.sync.dma_start(out=t_, in_=wk8d[d * P:(d + 1) * P, :])
                wk.append(t_)
            # qt: zero the other head's rows (t=0) and the whole t=1 plane;
            # kt: zero only the t=1 plane (t=0 is fully cast-written).
            # Ordered head-by-head so S(0, h) unblocks progressively.
            for h in range(HL):
                z0 = (1 - h % 2) * 64
                nc.sync.dma_start(out=qt[h][z0:z0 + 64, 0:T],
                                  in_=zerod[z0:z0 + 64, 0:T])
                nc.sync.dma_start(out=qt[h][:, T:2 * T], in_=zerod[:, 0:T])
                if h % 2 == 1:
                    m = h // 2
                    nc.sync.dma_start(out=kt[m][:, T:2 * T], in_=zerod[:, 0:T])
            for c in range(CCH):
                nc.sync.dma_start(out=xb16[c][:, 0:TB],
                                  in_=xb16d[c * P:(c + 1) * P, 0:TB])
                t_ = wp.tile([P, OC], bf16, tag=f"wv{c}", name=f"wv{c}")
                nc.sync.dma_start(out=t_, in_=wv16d[c * P:(c + 1) * P, :])
                wv.append(t_)
            # remaining token blocks of x (needed from proj(1) onward)
            for d in range(DCH):
                nc.sync.dma_start(out=xq8v[d][:, :, TB:T],
                                  in_=xq8dv[d * P:(d + 1) * P, :, TB:T])
            for c in range(CCH):
                nc.sync.dma_start(out=xb16[c][:, TB:T],
                                  in_=xb16d[c * P:(c + 1) * P, TB:T])
            for ci in range(MCH):
                t_ = wp.tile([P, C], bf16, tag=f"wo{ci}", name=f"wo{ci}")
                nc.sync.dma_start(out=t_, in_=wo16d[ci * P:(ci + 1) * P, :])
                wo.append(t_)

            def project_groups(tb):
                groups = []

                def proj_qk(w, isq, m, tb=tb):
                    def go():
                        ps = pp.tile([P, TB], f32, tag="mm512",
                                     name=f"ps_{tb}_{m}_{isq}")
                        for d in range(DCH):
                            nc.tensor.matmul(
                                ps,
                                lhsT=w[d].rearrange("p (t o) -> p t o", t=2)[
                                    :, :, m * P:(m + 1) * P],
                                rhs=xq8[d].rearrange("p (t f) -> p t f", t=2)[
                                    :, :, tb * TB:(tb + 1) * TB],
                                start=(d == 0), stop=(d == DCH - 1),
                                perf_mode=DR)
                        if isq:
                            # write each head's rows into its own qt tile
                            for hh in (0, 1):
                                r0_ = hh * 64
                                nc.vector.tensor_copy(
                                    qt[2 * m + hh][r0_:r0_ + 64,
                                                   tb * TB:(tb + 1) * TB],
                                    ps[r0_:r0_ + 64, :])
                        else:
                            nc.vector.tensor_copy(
                                kt[m][:, tb * TB:(tb + 1) * TB], ps)
                    return go

                def proj_v(ts_, tb=tb):
                    def go():
                        ps = pp.tile([P, OC], f32, tag="mm512",
                                     name=f"psv_{tb}_{ts_}")
                        for c in range(CCH):
                            nc.tensor.matmul(
                                ps,
                                lhsT=xb16[c][:, tb * TB + ts_ * P:
                                             tb * TB + (ts_ + 1) * P],
                                rhs=wv[c],
                                start=(c == 0), stop=(c == CCH - 1))
                        ti = tb * (TB // P) + ts_
                        nc.scalar.copy(
                            v[ti].rearrange("p (h e) -> p h e", e=VSTRIDE)[:, :, 0:64],
                            ps.rearrange("p (h d) -> p h d", d=64))
                    return go

                # zip Q and K per m-chunk so early heads' S deps land first
                for m in range(MCH):
                    groups.append(proj_qk(wq, True, m))
                    groups.append(proj_qk(wk, False, m))
                for ts_ in range(TB // P):
                    groups.append(proj_v(ts_))
                return groups

            def output_groups(tb):
                def out_co(co, tb=tb):
                    def go():
                        ps = pp.tile([P, TB], f32, tag="mm512",
                                     name=f"yps_{co}_{tb}")
                        for ci in range(MCH):
                            nc.tensor.matmul(
                                ps, lhsT=wo[ci][:, co * P:(co + 1) * P],
                                rhs=ct[ci][:, tb * TB:(tb + 1) * TB],
                                start=(ci == 0), stop=(ci == MCH - 1))
                        ysb = yp.tile([P, TB], bf16, tag="ysb", name=f"ysb_{co}_{tb}")
                        nc.vector.tensor_copy(ysb, ps)
                        nc.sync.dma_start(
                            out=yT[co * P:(co + 1) * P, tb * TB:(tb + 1) * TB],
                            in_=ysb)
                    return go
                return [out_co(co) for co in range(C // P)]

            pending = []

            def mk_norm(h, j, m, r0, ctx_ps):
                def norm():
                    s_sb = tmp.tile([1, TB], f32, tag="s", bufs=1, name=f"s_{h}_{j}")
                    nc.vector.tensor_copy(s_sb, ctx_ps[64:65, :])
                    rb = tmp.tile([64, TB], f32, tag="rb", bufs=1, name=f"rb_{h}_{j}")
                    nc.gpsimd.partition_broadcast(rb, s_sb)
                    nc.vector.reciprocal_approx_fast(out=rb, in_=rb)
                    nc.vector.tensor_mul(
                        ct[m][r0:r0 + 64, j * TB:(j + 1) * TB], ctx_ps[0:64, :], rb)
                return norm

            def attend(j, ilq):
                reserve = ilq[-2:]
                main = ilq[:max(0, len(ilq) - 2)]
                nch = 4 * (j + 1)
                npair = nch // 2

                def mk_ctx_chunks(h, pts):
                    # ctx matmuls for head h as small closures, ascending pair
                    # order (pair 0's chunk 0 covers the full q range so its
                    # start=True zeroes the whole psum). The psum tile is
                    # allocated lazily by the first closure.
                    m, r0 = h // 2, (h % 2) * 64
                    box = [None]
                    out = []

                    def pair_go(pp_, h=h, m=m):
                        def go():
                            if box[0] is None:
                                box[0] = cxp.tile([P, TB], f32, tag="ctx",
                                                  name=f"cps_{h}_{j}")
                            for t in (0, 1):
                                c = 2 * pp_ + t
                                qlo = max(0, 128 * c - TB * j)
                                nmm = 2 * pp_ + t
                                nc.tensor.matmul(
                                    box[0][0:65, qlo:TB],
                                    lhsT=v[c][:, h * VSTRIDE:h * VSTRIDE + 65],
                                    rhs=pts[pp_][:, t * TB + qlo:(t + 1) * TB],
                                    start=(nmm == 0), stop=(nmm == nch - 1),
                                    skip_group_check=True)
                        return go

                    for pp_ in range(npair):
                        out.append(pair_go(pp_))
                    out.append(lambda: pending.append(
                        mk_norm(h, j, h // 2, (h % 2) * 64, box[0])))
                    return out

                ctxq = []
                for h in range(HL):
                    m = h // 2
                    ktv = kt[m].rearrange("p (t f) -> p t f", t=2)
                    qtv = qt[h].rearrange("p (t f) -> p t f", t=2)
                    pts = [None] * npair
                    # S + exp + mask, diagonal pairs first (their longer
                    # exp->mask chain overlaps later S matmuls); previous
                    # head's ctx chunks fill the PE while S stalls on the
                    # st pool (exp rate-bound)
                    for pp_ in range(npair - 1, -1, -1):
                        st = stp.tile([P, 2 * TB], f32, tag="st",
                                      name=f"st_{h}_{j}_{pp_}")
                        qlo_pair = max(0, 128 * 2 * pp_ - TB * j)
                        for t in (0, 1):
                            c = 2 * pp_ + t
                            qlo = max(0, 128 * c - TB * j)
                            nc.tensor.matmul(
                                st[:, t * TB + qlo:(t + 1) * TB],
                                lhsT=ktv[:, :, c * P:(c + 1) * P],
                                rhs=qtv[:, :, j * TB + qlo:(j + 1) * TB],
                                start=True, stop=True, skip_group_check=True,
                                perf_mode=DR)
                        pt_ = ptp.tile([P, 2 * TB], bf16, tag="pt",
                                       name=f"pt_{h}_{j}_{pp_}")
                        pt3 = pt_.rearrange("p (t f) -> p t f", t=2)
                        st3 = st.rearrange("p (t f) -> p t f", t=2)
                        nc.scalar.activation(
                            pt3[:, :, qlo_pair:], st3[:, :, qlo_pair:],
                            EXP, scale=ESCALE)
                        if 2 * pp_ + 1 >= 4 * j:
                            # diagonal pair: tri-mask each chunk's 128-wide
                            # diagonal square (cols past it are all-keep,
                            # cols before it are outside the chunk's
                            # restricted ctx read range); on gpsimd - DVE is
                            # the busier gate for S/ctx deps
                            for t in (0, 1):
                                off = t * TB + 128 * (2 * pp_ + t) - TB * j
                                nc.gpsimd.tensor_mul(
                                    pt_[:, off:off + 128],
                                    pt_[:, off:off + 128], tri)
                        pts[pp_] = pt_
                        for _ in range(2):
                            if ctxq:
                                ctxq.pop(0)()
                            elif main:
                                main.pop(0)()
                                break
                    while ctxq:
                        ctxq.pop(0)()
                    if pending and h >= 2:
                        pending.pop(0)()
                    ctxq = mk_ctx_chunks(h, pts)
                while ctxq:
                    ctxq.pop(0)()
                for g in main + reserve:
                    g()
                while pending:
                    pending.pop(0)()

            for g in project_groups(0):
                g()
            for tb in range(NTB):
                ilq = []
                if tb + 1 < NTB:
                    ilq += project_groups(tb + 1)
                if tb >= 1:
                    ilq += output_groups(tb - 1)
                attend(tb, ilq)
            for g in output_groups(NTB - 1):
                g()

    nc.compile()
    return nc


def make_in_maps(x, Wq, Wk, Wv, Wo):
    import ml_dtypes

    f8 = ml_dtypes.float8_e4m3
    b16 = ml_dtypes.bfloat16
    x = np.asarray(x, np.float32)
    Wq, Wk, Wv, Wo = (np.asarray(w, np.float32) for w in (Wq, Wk, Wv, Wo))

    def dr_layout(a):
        # [C, N] -> DoubleRow fp8 [C/2, 2*N]: channel ch = 256*dc + 128*t + p
        Cd, N = a.shape
        return np.ascontiguousarray(
            a.reshape(Cd // 256, 2, 128, N).transpose(0, 2, 1, 3)
            .reshape(Cd // 2, 2 * N).astype(f8))

    tri = (np.arange(128)[None, :] >= np.arange(128)[:, None]).astype(b16)
    zero8 = np.zeros((128, T_FULL), f8)

    in_maps = []
    for core in range(NCORES):
        b, g = divmod(core, GROUPS)
        sl = slice(g * OC, (g + 1) * OC)
        xT = np.ascontiguousarray(x[b].T)
        in_maps.append({
            "xq8": dr_layout(xT),
            "xb16": xT.astype(b16),
            "wq8": dr_layout(np.ascontiguousarray(Wq[sl, :].T) * WSCALE),
            "wk8": dr_layout(np.ascontiguousarray(Wk[sl, :].T) * WSCALE),
            "wv16": np.ascontiguousarray(Wv[sl, :].T).astype(b16),
            "wo16": np.ascontiguousarray(Wo[:, sl].T).astype(b16),
            "tri": tri,
            "zero8": zero8,
        })
    return in_maps


def _run(inputs, trace=False):
    from concourse.bass_utils import run_bass_kernel_spmd

    nc = build_program()
    in_maps = make_in_maps(
        inputs["x"], inputs["Wq"], inputs["Wk"], inputs["Wv"], inputs["Wo"])
    res = run_bass_kernel_spmd(nc, in_maps, core_ids=list(range(NCORES)), trace=trace)
    y = np.zeros((B, T_FULL, C), np.float32)
    for core in range(NCORES):
        y[core // GROUPS] += res.results[core]["yT"].astype(np.float32).T
    return y, res


def kernel(**inputs):
    y, _ = _run(inputs)
    return y


# revision 19
# speedup vs baseline: 1.7666x; 1.7666x over previous
"""Causal multi-head attention (B=4, T=2048, C=1024, H=16, HD=64) on 8 TRN2
NeuronCores.

Sharding: 2D - batch (4) x head-group (2 groups of 8 heads). Each core handles
one batch's tokens for 8 heads (OC = 512 local channels); host sums the two
group partials of y.

v2 layout/precision strategy:
  - Q/K projections run in fp8e4 DoubleRow mode (x and 32*Wq/32*Wk quantized
    to fp8 on host, contraction 256/instr at 0.5 cyc/row). The 32*32=1024
    factor is folded into the exp scale.
  - Q^T/K^T stored as fp8 [128, 2, T] DoubleRow operands (t=1 plane zeroed,
    qt additionally zero on the other head's 64 rows); S^T chunks
    [k=128, q<=512] via fp8 DoubleRow at 0.5 cyc/row.
  - V projection and output projection in bf16 (1 cyc/row).
  - Causal q-range restriction at 128-granularity on S, exp, and ctx
    (diagonal chunks only compute q >= 128*kc).
  - Masking: DVE multiplies with host-provided lower-tri mask tiles (T0/T1)
    on the two diagonal chunks per (head, block) - gpsimd only does the
    denominator partition_broadcast and v-ones memsets.
  - ctx matmuls bf16 with the [V_h | 1] ones-column denominator trick;
    normalize via broadcast + DVE reciprocal + fused mul into bf16 ct.
  - y^T computed in bf16, host converts/sums in f32.
"""

import numpy as np

B, T_FULL, C = 4, 2048, 1024
H, HD = 16, 64
GROUPS = 2
HL = H // GROUPS          # heads per core = 8
OC = HL * HD              # local channels = 512
P = 128                   # partitions
TB = 512                  # token block (moving dim)
SCALE = float(1.0 / np.sqrt(HD))
NCORES = 8
WSCALE = 32.0             # host premultiplies Wq/Wk before fp8 quantization
VSTRIDE = 72              # per-head column stride in v tiles (64 d + 1 one + 7 pad)


def build_program(T=T_FULL):
    from contextlib import ExitStack

    import concourse.bacc as bacc
    import concourse.mybir as mybir
    import concourse.tile as tile

    f32 = mybir.dt.float32
    bf16 = mybir.dt.bfloat16
    fp8 = mybir.dt.float8e4
    EXP = mybir.ActivationFunctionType.Exp
    DR = mybir.MatmulPerfMode.DoubleRow
    ESCALE = SCALE / (WSCALE * WSCALE)

    NTB = T // TB             # 512-token blocks
    NKC = T // P              # 128-token key chunks
    CCH = C // P              # 8 contraction chunks of C
    DCH = C // (2 * P)        # 4 double-contraction chunks (fp8 DoubleRow)
    MCH = OC // P             # 4 output-channel chunks

    nc = bacc.Bacc("TRN2", target_bir_lowering=False, debug=False)
    xq8d = nc.dram_tensor("xq8", [C // 2, 2 * T], fp8, kind="ExternalInput").ap()
    xb16d = nc.dram_tensor("xb16", [C, T], bf16, kind="ExternalInput").ap()
    wq8d = nc.dram_tensor("wq8", [C // 2, 2 * OC], fp8, kind="ExternalInput").ap()
    wk8d = nc.dram_tensor("wk8", [C // 2, 2 * OC], fp8, kind="ExternalInput").ap()
    wv16d = nc.dram_tensor("wv16", [C, OC], bf16, kind="ExternalInput").ap()
    wo16d = nc.dram_tensor("wo16", [OC, C], bf16, kind="ExternalInput").ap()
    trid = nc.dram_tensor("tri", [P, P], bf16, kind="ExternalInput").ap()
    zerod = nc.dram_tensor("zero8", [P, T], fp8, kind="ExternalInput").ap()
    yT = nc.dram_tensor("yT", [C, T], bf16, kind="ExternalOutput").ap()

    with tile.TileContext(nc) as tc, ExitStack() as ctx:
        perm = ctx.enter_context(tc.tile_pool(name="perm", bufs=1))
        # per-head Q^T fp8 [128, 2, T]: rows (h%2)*64..+64 of t=0 hold the
        # head's channels; everything else stays zero (zero-DMA'd once) so the
        # DoubleRow contraction only sees this head's 64 channels.
        qt = [perm.tile([P, 2 * T], fp8, tag=f"qt{h}", name=f"qt{h}")
              for h in range(HL)]
        # per-pair K^T fp8 [128, 2, T]: t=0 holds both heads' channels
        # (rows = oc chunk), t=1 zero.
        kt = [perm.tile([P, 2 * T], fp8, tag=f"kt{m}", name=f"kt{m}")
              for m in range(MCH)]
        # normalized ctx^T bf16 [oc rows, T] per m-chunk
        ct = [perm.tile([P, T], bf16, tag=f"ct{m}", name=f"ct{m}") for m in range(MCH)]
        # V bf16 per key chunk: head stride 72 = [64 vals | 1 | 7 junk]
        v = [perm.tile([P, HL * VSTRIDE], bf16, tag=f"v{t}", name=f"v{t}")
             for t in range(NKC)]
        tri = perm.tile([P, P], bf16, tag="tri", name="tri")
        ONE_BF16 = 0x3F80
        for vt in v:
            vv = vt.rearrange("p (h e) -> p h e", e=VSTRIDE)
            nc.gpsimd.memset(vv[:, :, 64:65].bitcast(mybir.dt.uint16), ONE_BF16)

        # x resident in SBUF for the whole kernel (48KB)
        xq8 = [perm.tile([P, 2 * T], fp8, tag=f"xq8_{d}", name=f"xq8_{d}")
               for d in range(DCH)]
        xb16 = [perm.tile([P, T], bf16, tag=f"xb{c}", name=f"xb{c}")
                for c in range(CCH)]

        with (
            tc.tile_pool(name="wpool", bufs=1) as wp,
            tc.tile_pool(name="ptpool", bufs=16) as ptp,
            tc.tile_pool(name="tmppool", bufs=2) as tmp,
            tc.tile_pool(name="ypool", bufs=2) as yp,
            tc.tile_pool(name="mmps", bufs=2, space="PSUM") as pp,
            tc.tile_pool(name="stps", bufs=2, space="PSUM") as stp,
            tc.tile_pool(name="ctxps", bufs=2, space="PSUM") as cxp,
        ):
            # ---- loads, ordered so block-0's dependency chain lands first:
            # tri; xq8/wq8/wk8 block-0 slices; per-tile zero planes (gate the
            # first casts / S matmuls); xb16/wv block-0; the rest streams in
            # behind while attend(0) runs.
            nc.sync.dma_start(out=tri, in_=trid)
            wq, wk, wv, wo = [], [], [], []
            xq8v = [x_.rearrange("p (t f) -> p t f", t=2) for x_ in xq8]
            xq8dv = xq8d.rearrange("p (t f) -> p t f", t=2)
            for d in range(DCH):
                nc.sync.dma_start(out=xq8v[d][:, :, 0:TB],
                                  in_=xq8dv[d * P:(d + 1) * P, :, 0:TB])
                t_ = wp.tile([P, 2 * OC], fp8, tag=f"wq{d}", name=f"wq{d}")
                nc.sync.dma_start(out=t_, in_=wq8d[d * P:(d + 1) * P, :])
                wq.append(t_)
            for d in range(DCH):
                t_ = wp.tile([P, 2 * OC], fp8, tag=f"wk{d}", name=f"wk{d}")
                nc.sync.dma_start(out=t_, in_=wk8d[d * P:(d + 1) * P, :])
                wk.append(t_)
            # qt: zero the other head's rows (t=0) and the whole t=1 plane;
            # kt: zero only the t=1 plane (t=0 is fully cast-written).
            # Ordered head-by-head so S(0, h) unblocks progressively.
            for h in range(HL):
                z0 = (1 - h % 2) * 64
                nc.sync.dma_start(out=qt[h][z0:z0 + 64, 0:T],
                                  in_=zerod[z0:z0 + 64, 0:T])
                nc.sync.dma_start(out=qt[h][:, T:2 * T], in_=zerod[:, 0:T])
                if h % 2 == 1:
                    m = h // 2
                    nc.sync.dma_start(out=kt[m][:, T:2 * T], in_=zerod[:, 0:T])
            for c in range(CCH):
                nc.sync.dma_start(out=xb16[c][:, 0:TB],
                                  in_=xb16d[c * P:(c + 1) * P, 0:TB])
                t_ = wp.tile([P, OC], bf16, tag=f"wv{c}", name=f"wv{c}")
                nc.sync.dma_start(out=t_, in_=wv16d[c * P:(c + 1) * P, :])
                wv.append(t_)
            # remaining token blocks of x (needed from proj(1) onward)
            for d in range(DCH):
                nc.sync.dma_start(out=xq8v[d][:, :, TB:T],
                                  in_=xq8dv[d * P:(d + 1) * P, :, TB:T])
            for c in range(CCH):
                nc.sync.dma_start(out=xb16[c][:, TB:T],
                                  in_=xb16d[c * P:(c + 1) * P, TB:T])
            for ci in range(MCH):
                t_ = wp.tile([P, C], bf16, tag=f"wo{ci}", name=f"wo{ci}")
                nc.sync.dma_start(out=t_, in_=wo16d[ci * P:(ci + 1) * P, :])
                wo.append(t_)

            def project_groups(tb):
                groups = []

                def proj_qk(w, isq, m, tb=tb):
                    def go():
                        ps = pp.tile([P, TB], f32, tag="mm512",
                                     name=f"ps_{tb}_{m}_{isq}")
                        for d in range(DCH):
                            nc.tensor.matmul(
                                ps,
                                lhsT=w[d].rearrange("p (t o) -> p t o", t=2)[
                                    :, :, m * P:(m + 1) * P],
                                rhs=xq8[d].rearrange("p (t f) -> p t f", t=2)[
                                    :, :, tb * TB:(tb + 1) * TB],
                                start=(d == 0), stop=(d == DCH - 1),
                                perf_mode=DR)
                        if isq:
                            # write each head's rows into its own qt tile
                            for hh in (0, 1):
                                r0_ = hh * 64
                                nc.vector.tensor_copy(
                                    qt[2 * m + hh][r0_:r0_ + 64,
                                                   tb * TB:(tb + 1) * TB],
                                    ps[r0_:r0_ + 64, :])
                        else:
                            nc.vector.tensor_copy(
                                kt[m][:, tb * TB:(tb + 1) * TB], ps)
                    return go

                def proj_v(ts_, tb=tb):
                    def go():
                        ps = pp.tile([P, OC], f32, tag="mm512",
                                     name=f"psv_{tb}_{ts_}")
                        for c in range(CCH):
                            nc.tensor.matmul(
                                ps,
                                lhsT=xb16[c][:, tb * TB + ts_ * P:
                                             tb * TB + (ts_ + 1) * P],
                                rhs=wv[c],
                                start=(c == 0), stop=(c == CCH - 1))
                        ti = tb * (TB // P) + ts_
                        nc.scalar.copy(
                            v[ti].rearrange("p (h e) -> p h e", e=VSTRIDE)[:, :, 0:64],
                            ps.rearrange("p (h d) -> p h d", d=64))
                    return go

                # zip Q and K per m-chunk so early heads' S deps land first
                for m in range(MCH):
                    groups.append(proj_qk(wq, True, m))
                    groups.append(proj_qk(wk, False, m))
                for ts_ in range(TB // P):
                    groups.append(proj_v(ts_))
                return groups

            def output_groups(tb):
                def out_co(co, tb=tb):
                    def go():
                        ps = pp.tile([P, TB], f32, tag="mm512",
                                     name=f"yps_{co}_{tb}")
                        for ci in range(MCH):
                            nc.tensor.matmul(
                                ps, lhsT=wo[ci][:, co * P:(co + 1) * P],
                                rhs=ct[ci][:, tb * TB:(tb + 1) * TB],
                                start=(ci == 0), stop=(ci == MCH - 1))
                        ysb = yp.tile([P, TB], bf16, tag="ysb", name=f"ysb_{co}_{tb}")
                        nc.vector.tensor_copy(ysb, ps)
                        nc.sync.dma_start(
                            out=yT[co * P:(co + 1) * P, tb * TB:(tb + 1) * TB],
                            in_=ysb)
                    return go
                return [out_co(co) for co in range(C // P)]

            pending = []

            def mk_norm(h, j, m, r0, ctx_ps):
                def norm():
                    s_sb = tmp.tile([1, TB], f32, tag="s", bufs=1, name=f"s_{h}_{j}")
                    nc.vector.tensor_copy(s_sb, ctx_ps[64:65, :])
                    rb = tmp.tile([64, TB], f32, tag="rb", bufs=1, name=f"rb_{h}_{j}")
                    nc.gpsimd.partition_broadcast(rb, s_sb)
                    nc.vector.reciprocal_approx_fast(out=rb, in_=rb)
                    nc.vector.tensor_mul(
                        ct[m][r0:r0 + 64, j * TB:(j + 1) * TB], ctx_ps[0:64, :], rb)
                return norm

            def attend(j, ilq):
                reserve = ilq[-2:]
                main = ilq[:max(0, len(ilq) - 2)]
                nch = 4 * (j + 1)
                npair = nch // 2

                def mk_ctx_chunks(h, pts):
                    # ctx matmuls for head h as small closures, ascending pair
                    # order (pair 0's chunk 0 covers the full q range so its
                    # start=True zeroes the whole psum). The psum tile is
                    # allocated lazily by the first closure.
                    m, r0 = h // 2, (h % 2) * 64
                    box = [None]
                    out = []

                    def pair_go(pp_, h=h, m=m):
                        def go():
                            if box[0] is None:
                                box[0] = cxp.tile([P, TB], f32, tag="ctx",
                                                  name=f"cps_{h}_{j}")
                            for t in (0, 1):
                                c = 2 * pp_ + t
                                qlo = max(0, 128 * c - TB * j)
                                nmm = 2 * pp_ + t
                                nc.tensor.matmul(
                                    box[0][0:65, qlo:TB],
                                    lhsT=v[c][:, h * VSTRIDE:h * VSTRIDE + 65],
                                    rhs=pts[pp_][:, t * TB + qlo:(t + 1) * TB],
                                    start=(nmm == 0), stop=(nmm == nch - 1),
                                    skip_group_check=True)
                        return go

                    for pp_ in range(npair):
                        out.append(pair_go(pp_))
                    out.append(lambda: pending.append(
                        mk_norm(h, j, h // 2, (h % 2) * 64, box[0])))
                    return out

                ctxq = []
                for h in range(HL):
                    m = h // 2
                    ktv = kt[m].rearrange("p (t f) -> p t f", t=2)
                    qtv = qt[h].rearrange("p (t f) -> p t f", t=2)
                    pts = [None] * npair
                    # S + exp + mask, diagonal pairs first (their longer
                    # exp->mask chain overlaps later S matmuls); previous
                    # head's ctx chunks fill the PE while S stalls on the
                    # st pool (exp rate-bound)
                    for pp_ in range(npair - 1, -1, -1):
                        st = stp.tile([P, 2 * TB], f32, tag="st",
                                      name=f"st_{h}_{j}_{pp_}")
                        qlo_pair = max(0, 128 * 2 * pp_ - TB * j)
                        for t in (0, 1):
                            c = 2 * pp_ + t
                            qlo = max(0, 128 * c - TB * j)
                            nc.tensor.matmul(
                                st[:, t * TB + qlo:(t + 1) * TB],
                                lhsT=ktv[:, :, c * P:(c + 1) * P],
                                rhs=qtv[:, :, j * TB + qlo:(j + 1) * TB],
                                start=True, stop=True, skip_group_check=True,
                                perf_mode=DR)
                        pt_ = ptp.tile([P, 2 * TB], bf16, tag="pt",
                                       name=f"pt_{h}_{j}_{pp_}")
                        pt3 = pt_.rearrange("p (t f) -> p t f", t=2)
                        st3 = st.rearrange("p (t f) -> p t f", t=2)
                        nc.scalar.activation(
                            pt3[:, :, qlo_pair:], st3[:, :, qlo_pair:],
                            EXP, scale=ESCALE)
                        if 2 * pp_ + 1 >= 4 * j:
                            # diagonal pair: tri-mask each chunk's 128-wide
                            # diagonal square (cols past it are all-keep,
                            # cols before it are outside the chunk's
                            # restricted ctx read range); keep on DVE - the
                            # gpsimd queue is in-order and slow, serializing
                            # this exp->mask->ctx critical chain
                            for t in (0, 1):
                                off = t * TB + 128 * (2 * pp_ + t) - TB * j
                                nc.vector.tensor_mul(
                                    pt_[:, off:off + 128],
                                    pt_[:, off:off + 128], tri)
                        pts[pp_] = pt_
                        for _ in range(2):
                            if ctxq:
                                ctxq.pop(0)()
                            elif main:
                                main.pop(0)()
                                break
                    while ctxq:
                        ctxq.pop(0)()
                    if pending and h >= 2:
                        pending.pop(0)()
                    ctxq = mk_ctx_chunks(h, pts)
                while ctxq:
                    ctxq.pop(0)()
                for g in main + reserve:
                    g()
                while pending:
                    pending.pop(0)()

            for g in project_groups(0):
                g()
            for tb in range(NTB):
                ilq = []
                if tb + 1 < NTB:
                    ilq += project_groups(tb + 1)
                if tb >= 1:
                    ilq += output_groups(tb - 1)
                attend(tb, ilq)
            for g in output_groups(NTB - 1):
                g()

    nc.compile()
    return nc


def make_in_maps(x, Wq, Wk, Wv, Wo):
    import ml_dtypes

    f8 = ml_dtypes.float8_e4m3
    b16 = ml_dtypes.bfloat16
    x = np.asarray(x, np.float32)
    Wq, Wk, Wv, Wo = (np.asarray(w, np.float32) for w in (Wq, Wk, Wv, Wo))

    def dr_layout(a):
        # [C, N] -> DoubleRow fp8 [C/2, 2*N]: channel ch = 256*dc + 128*t + p
        Cd, N = a.shape
        return np.ascontiguousarray(
            a.reshape(Cd // 256, 2, 128, N).transpose(0, 2, 1, 3)
            .reshape(Cd // 2, 2 * N).astype(f8))

    tri = (np.arange(128)[None, :] >= np.arange(128)[:, None]).astype(b16)
    zero8 = np.zeros((128, T_FULL), f8)

    in_maps = []
    for core in range(NCORES):
        b, g = divmod(core, GROUPS)
        sl = slice(g * OC, (g + 1) * OC)
        xT = np.ascontiguousarray(x[b].T)
        in_maps.append({
            "xq8": dr_layout(xT),
            "xb16": xT.astype(b16),
            "wq8": dr_layout(np.ascontiguousarray(Wq[sl, :].T) * WSCALE),
            "wk8": dr_layout(np.ascontiguousarray(Wk[sl, :].T) * WSCALE),
            "wv16": np.ascontiguousarray(Wv[sl, :].T).astype(b16),
            "wo16": np.ascontiguousarray(Wo[:, sl].T).astype(b16),
            "tri": tri,
            "zero8": zero8,
        })
    return in_maps


def _run(inputs, trace=False):
    from concourse.bass_utils import run_bass_kernel_spmd

    nc = build_program()
    in_maps = make_in_maps(
        inputs["x"], inputs["Wq"], inputs["Wk"], inputs["Wv"], inputs["Wo"])
    res = run_bass_kernel_spmd(nc, in_maps, core_ids=list(range(NCORES)), trace=trace)
    y = np.zeros((B, T_FULL, C), np.float32)
    for core in range(NCORES):
        y[core // GROUPS] += res.results[core]["yT"].astype(np.float32).T
    return y, res


def kernel(**inputs):
    y, _ = _run(inputs)
    return y


# revision 24
# speedup vs baseline: 1.7884x; 1.0123x over previous
"""Causal multi-head attention (B=4, T=2048, C=1024, H=16, HD=64) on 8 TRN2
NeuronCores.

Sharding: 2D - batch (4) x head-group (2 groups of 8 heads). Each core handles
one batch's tokens for 8 heads (OC = 512 local channels); host sums the two
group partials of y.

v2 layout/precision strategy:
  - Q/K projections run in fp8e4 DoubleRow mode (x and 32*Wq/32*Wk quantized
    to fp8 on host, contraction 256/instr at 0.5 cyc/row). The 32*32=1024
    factor is folded into the exp scale.
  - Q^T/K^T stored as fp8 [128, 2, T] DoubleRow operands (t=1 plane zeroed,
    qt additionally zero on the other head's 64 rows); S^T chunks
    [k=128, q<=512] via fp8 DoubleRow at 0.5 cyc/row.
  - V projection and output projection in bf16 (1 cyc/row).
  - Causal q-range restriction at 128-granularity on S, exp, and ctx
    (diagonal chunks only compute q >= 128*kc).
  - Masking: DVE multiplies with host-provided lower-tri mask tiles (T0/T1)
    on the two diagonal chunks per (head, block) - gpsimd only does the
    denominator partition_broadcast and v-ones memsets.
  - ctx matmuls bf16 with the [V_h | 1] ones-column denominator trick;
    normalize via broadcast + DVE reciprocal + fused mul into bf16 ct.
  - y^T computed in bf16, host converts/sums in f32.
"""

import numpy as np

B, T_FULL, C = 4, 2048, 1024
H, HD = 16, 64
GROUPS = 2
HL = H // GROUPS          # heads per core = 8
OC = HL * HD              # local channels = 512
P = 128                   # partitions
TB = 512                  # token block (moving dim)
SCALE = float(1.0 / np.sqrt(HD))
NCORES = 8
WSCALE = 32.0             # host premultiplies Wq/Wk before fp8 quantization
VSTRIDE = 72              # per-head column stride in v tiles (64 d + 1 one + 7 pad)


def build_program(T=T_FULL):
    from contextlib import ExitStack

    import concourse.bacc as bacc
    import concourse.mybir as mybir
    import concourse.tile as tile

    f32 = mybir.dt.float32
    bf16 = mybir.dt.bfloat16
    fp8 = mybir.dt.float8e4
    EXP = mybir.ActivationFunctionType.Exp
    DR = mybir.MatmulPerfMode.DoubleRow
    ESCALE = SCALE / (WSCALE * WSCALE)

    NTB = T // TB             # 512-token blocks
    NKC = T // P              # 128-token key chunks
    CCH = C // P              # 8 contraction chunks of C
    DCH = C // (2 * P)        # 4 double-contraction chunks (fp8 DoubleRow)
    MCH = OC // P             # 4 output-channel chunks

    nc = bacc.Bacc("TRN2", target_bir_lowering=False, debug=False)
    xq8d = nc.dram_tensor("xq8", [C // 2, 2 * T], fp8, kind="ExternalInput").ap()
    xb16d = nc.dram_tensor("xb16", [C, T], bf16, kind="ExternalInput").ap()
    wq8d = nc.dram_tensor("wq8", [C // 2, 2 * OC], fp8, kind="ExternalInput").ap()
    wk8d = nc.dram_tensor("wk8", [C // 2, 2 * OC], fp8, kind="ExternalInput").ap()
    wv16d = nc.dram_tensor("wv16", [C, OC], bf16, kind="ExternalInput").ap()
    wo16d = nc.dram_tensor("wo16", [OC, C], bf16, kind="ExternalInput").ap()
    trid = nc.dram_tensor("tri", [P, P], bf16, kind="ExternalInput").ap()
    zerod = nc.dram_tensor("zero8", [P, T], fp8, kind="ExternalInput").ap()
    yT = nc.dram_tensor("yT", [C, T], bf16, kind="ExternalOutput").ap()

    with tile.TileContext(nc) as tc, ExitStack() as ctx:
        perm = ctx.enter_context(tc.tile_pool(name="perm", bufs=1))
        # per-head Q^T fp8 [128, 2, T]: rows (h%2)*64..+64 of t=0 hold the
        # head's channels; everything else stays zero (zero-DMA'd once) so the
        # DoubleRow contraction only sees this head's 64 channels.
        qt = [perm.tile([P, 2 * T], fp8, tag=f"qt{h}", name=f"qt{h}")
              for h in range(HL)]
        # per-pair K^T fp8 [128, 2, T]: t=0 holds both heads' channels
        # (rows = oc chunk), t=1 zero.
        kt = [perm.tile([P, 2 * T], fp8, tag=f"kt{m}", name=f"kt{m}")
              for m in range(MCH)]
        # normalized ctx^T bf16 [oc rows, T] per m-chunk
        ct = [perm.tile([P, T], bf16, tag=f"ct{m}", name=f"ct{m}") for m in range(MCH)]
        # V bf16 per key chunk: head stride 72 = [64 vals | 1 | 7 junk]
        v = [perm.tile([P, HL * VSTRIDE], bf16, tag=f"v{t}", name=f"v{t}")
             for t in range(NKC)]
        tri = perm.tile([P, P], bf16, tag="tri", name="tri")
        ONE_BF16 = 0x3F80
        for vt in v:
            vv = vt.rearrange("p (h e) -> p h e", e=VSTRIDE)
            nc.gpsimd.memset(vv[:, :, 64:65].bitcast(mybir.dt.uint16), ONE_BF16)

        with (
            tc.tile_pool(name="wpool", bufs=1) as wp,
            tc.tile_pool(name="xqpool", bufs=2) as xqp,
            tc.tile_pool(name="xbpool", bufs=2) as xbp,
            tc.tile_pool(name="ptpool", bufs=32) as ptp,
            tc.tile_pool(name="tmppool", bufs=2) as tmp,
            tc.tile_pool(name="ypool", bufs=2) as yp,
            tc.tile_pool(name="mmps", bufs=2, space="PSUM") as pp,
            tc.tile_pool(name="stps", bufs=2, space="PSUM") as stp,
            tc.tile_pool(name="ctxps", bufs=2, space="PSUM") as cxp,
        ):
            xq8dv = xq8d.rearrange("p (t f) -> p t f", t=2)

            def load_x(tb):
                xq_t, xb_t = [], []
                for d in range(DCH):
                    t_ = xqp.tile([P, 2 * TB], fp8, tag=f"xq{d}",
                                  name=f"xq_{tb}_{d}")
                    nc.sync.dma_start(
                        out=t_.rearrange("p (t f) -> p t f", t=2),
                        in_=xq8dv[d * P:(d + 1) * P, :, tb * TB:(tb + 1) * TB])
                    xq_t.append(t_)
                for c in range(CCH):
                    t_ = xbp.tile([P, TB], bf16, tag=f"xb{c}",
                                  name=f"xb_{tb}_{c}")
                    nc.sync.dma_start(
                        out=t_, in_=xb16d[c * P:(c + 1) * P,
                                          tb * TB:(tb + 1) * TB])
                    xb_t.append(t_)
                return xq_t, xb_t

            # ---- loads, ordered so block-0's dependency chain lands first:
            # tri; x/wq8/wk8 block-0; per-tile zero planes (gate the first
            # casts / S matmuls); xb16/wv block-0; wo streams in behind.
            nc.sync.dma_start(out=tri, in_=trid)
            wq, wk, wv, wo = [], [], [], []
            xq_0 = []
            for d in range(DCH):
                t_ = xqp.tile([P, 2 * TB], fp8, tag=f"xq{d}", name=f"xq_0_{d}")
                nc.sync.dma_start(
                    out=t_.rearrange("p (t f) -> p t f", t=2),
                    in_=xq8dv[d * P:(d + 1) * P, :, 0:TB])
                xq_0.append(t_)
                t_ = wp.tile([P, 2 * OC], fp8, tag=f"wq{d}", name=f"wq{d}")
                nc.sync.dma_start(out=t_, in_=wq8d[d * P:(d + 1) * P, :])
                wq.append(t_)
            for d in range(DCH):
                t_ = wp.tile([P, 2 * OC], fp8, tag=f"wk{d}", name=f"wk{d}")
                nc.sync.dma_start(out=t_, in_=wk8d[d * P:(d + 1) * P, :])
                wk.append(t_)
            # qt: zero the other head's rows (t=0) and the whole t=1 plane;
            # kt: zero only the t=1 plane (t=0 is fully cast-written).
            for h in range(HL):
                z0 = (1 - h % 2) * 64
                nc.sync.dma_start(out=qt[h][z0:z0 + 64, 0:T],
                                  in_=zerod[z0:z0 + 64, 0:T])
                nc.sync.dma_start(out=qt[h][:, T:2 * T], in_=zerod[:, 0:T])
                if h % 2 == 1:
                    nc.sync.dma_start(out=kt[h // 2][:, T:2 * T],
                                      in_=zerod[:, 0:T])
            xb_0 = []
            for c in range(CCH):
                t_ = xbp.tile([P, TB], bf16, tag=f"xb{c}", name=f"xb_0_{c}")
                nc.sync.dma_start(out=t_, in_=xb16d[c * P:(c + 1) * P, 0:TB])
                xb_0.append(t_)
                t_ = wp.tile([P, OC], bf16, tag=f"wv{c}", name=f"wv{c}")
                nc.sync.dma_start(out=t_, in_=wv16d[c * P:(c + 1) * P, :])
                wv.append(t_)
            for ci in range(MCH):
                t_ = wp.tile([P, C], bf16, tag=f"wo{ci}", name=f"wo{ci}")
                nc.sync.dma_start(out=t_, in_=wo16d[ci * P:(ci + 1) * P, :])
                wo.append(t_)

            def project_groups(tb, xq_t, xb_t):
                groups = []

                def proj_qk(w, isq, m, tb=tb):
                    def go():
                        ps = pp.tile([P, TB], f32, tag="mm512",
                                     name=f"ps_{tb}_{m}_{isq}")
                        for d in range(DCH):
                            nc.tensor.matmul(
                                ps,
                                lhsT=w[d].rearrange("p (t o) -> p t o", t=2)[
                                    :, :, m * P:(m + 1) * P],
                                rhs=xq_t[d].rearrange("p (t f) -> p t f", t=2),
                                start=(d == 0), stop=(d == DCH - 1),
                                perf_mode=DR)
                        if isq:
                            # write each head's rows into its own qt tile
                            for hh in (0, 1):
                                r0_ = hh * 64
                                nc.vector.tensor_copy(
                                    qt[2 * m + hh][r0_:r0_ + 64,
                                                   tb * TB:(tb + 1) * TB],
                                    ps[r0_:r0_ + 64, :])
                        else:
                            nc.vector.tensor_copy(
                                kt[m][:, tb * TB:(tb + 1) * TB], ps)
                    return go

                def proj_v(ts_, tb=tb):
                    def go():
                        ps = pp.tile([P, OC], f32, tag="mm512",
                                     name=f"psv_{tb}_{ts_}")
                        for c in range(CCH):
                            nc.tensor.matmul(
                                ps,
                                lhsT=xb_t[c][:, ts_ * P:(ts_ + 1) * P],
                                rhs=wv[c],
                                start=(c == 0), stop=(c == CCH - 1))
                        ti = tb * (TB // P) + ts_
                        nc.vector.tensor_copy(
                            v[ti].rearrange("p (h e) -> p h e", e=VSTRIDE)[:, :, 0:64],
                            ps.rearrange("p (h d) -> p h d", d=64))
                    return go

                # zip Q and K per m-chunk so early heads' S deps land first
                for m in range(MCH):
                    groups.append(proj_qk(wq, True, m))
                    groups.append(proj_qk(wk, False, m))
                for ts_ in range(TB // P):
                    groups.append(proj_v(ts_))
                return groups

            def output_groups(tb):
                def out_co(co, tb=tb):
                    def go():
                        ps = pp.tile([P, TB], f32, tag="mm512",
                                     name=f"yps_{co}_{tb}")
                        for ci in range(MCH):
                            nc.tensor.matmul(
                                ps, lhsT=wo[ci][:, co * P:(co + 1) * P],
                                rhs=ct[ci][:, tb * TB:(tb + 1) * TB],
                                start=(ci == 0), stop=(ci == MCH - 1))
                        ysb = yp.tile([P, TB], bf16, tag="ysb", name=f"ysb_{co}_{tb}")
                        nc.vector.tensor_copy(ysb, ps)
                        nc.sync.dma_start(
                            out=yT[co * P:(co + 1) * P, tb * TB:(tb + 1) * TB],
                            in_=ysb)
                    return go
                return [out_co(co) for co in range(C // P)]

            pending = []

            def mk_norm(h, j, m, r0, ctx_ps):
                def norm():
                    s_sb = tmp.tile([1, TB], f32, tag="s", bufs=1, name=f"s_{h}_{j}")
                    nc.vector.tensor_copy(s_sb, ctx_ps[64:65, :])
                    rb = tmp.tile([64, TB], f32, tag="rb", bufs=1, name=f"rb_{h}_{j}")
                    nc.gpsimd.partition_broadcast(rb, s_sb)
                    nc.vector.reciprocal_approx_fast(out=rb, in_=rb)
                    nc.vector.tensor_mul(
                        ct[m][r0:r0 + 64, j * TB:(j + 1) * TB], ctx_ps[0:64, :], rb)
                return norm

            pts_store = {}

            def s_phase(j, h, filler):
                # S + exp + mask for head (j, h), diagonal pairs first (their
                # longer exp->mask chain overlaps later S matmuls); filler(k)
                # is called between pairs to keep the PE busy while S stalls
                # on the st pool (exp rate-bound).
                npair = 2 * (j + 1)
                m = h // 2
                ktv = kt[m].rearrange("p (t f) -> p t f", t=2)
                qtv = qt[h].rearrange("p (t f) -> p t f", t=2)
                pts = [None] * npair
                for pp_ in range(npair - 1, -1, -1):
                    st = stp.tile([P, 2 * TB], f32, tag="st",
                                  name=f"st_{h}_{j}_{pp_}")
                    qlo_pair = max(0, 128 * 2 * pp_ - TB * j)
                    for t in (0, 1):
                        c = 2 * pp_ + t
                        qlo = max(0, 128 * c - TB * j)
                        nc.tensor.matmul(
                            st[:, t * TB + qlo:(t + 1) * TB],
                            lhsT=ktv[:, :, c * P:(c + 1) * P],
                            rhs=qtv[:, :, j * TB + qlo:(j + 1) * TB],
                            start=True, stop=True, skip_group_check=True,
                            perf_mode=DR)
                    pt_ = ptp.tile([P, 2 * TB], bf16, tag="pt",
                                   name=f"pt_{h}_{j}_{pp_}")
                    pt3 = pt_.rearrange("p (t f) -> p t f", t=2)
                    st3 = st.rearrange("p (t f) -> p t f", t=2)
                    nc.scalar.activation(
                        pt3[:, :, qlo_pair:], st3[:, :, qlo_pair:],
                        EXP, scale=ESCALE)
                    if 2 * pp_ + 1 >= 4 * j:
                        # diagonal pair: tri-mask each chunk's 128-wide
                        # diagonal square (on DVE - the gpsimd queue is
                        # in-order and slow, serializing this chain)
                        for t in (0, 1):
                            off = t * TB + 128 * (2 * pp_ + t) - TB * j
                            nc.vector.tensor_mul(
                                pt_[:, off:off + 128],
                                pt_[:, off:off + 128], tri)
                    pts[pp_] = pt_
                    filler(2)
                return pts

            def mk_ctx_chunks(j, h, pts):
                # ctx matmuls for head h as small closures, ascending pair
                # order (pair 0's chunk 0 covers the full q range so its
                # start=True zeroes the whole psum). The psum tile is
                # allocated lazily by the first closure.
                nch = 4 * (j + 1)
                box = [None]
                out = []

                def pair_go(pp_):
                    def go():
                        if box[0] is None:
                            box[0] = cxp.tile([P, TB], f32, tag="ctx",
                                              name=f"cps_{h}_{j}")
                        for t in (0, 1):
                            c = 2 * pp_ + t
                            qlo = max(0, 128 * c - TB * j)
                            nmm = 2 * pp_ + t
                            nc.tensor.matmul(
                                box[0][0:65, qlo:TB],
                                lhsT=v[c][:, h * VSTRIDE:h * VSTRIDE + 65],
                                rhs=pts[pp_][:, t * TB + qlo:(t + 1) * TB],
                                start=(nmm == 0), stop=(nmm == nch - 1),
                                skip_group_check=True)
                    return go

                for pp_ in range(nch // 2):
                    out.append(pair_go(pp_))
                out.append(lambda: pending.append(
                    mk_norm(h, j, h // 2, (h % 2) * 64, box[0])))
                return out

            def attend(j, ilq, look):
                reserve = ilq[-2:]
                main = ilq[:max(0, len(ilq) - 2)]
                ctxq = []

                def filler(k):
                    for _ in range(k):
                        if ctxq:
                            ctxq.pop(0)()
                        elif main:
                            main.pop(0)()
                            return

                for h in range(HL):
                    pts = pts_store.pop((j, h), None)
                    if pts is None:
                        pts = s_phase(j, h, filler)
                    else:
                        filler(3)
                    while ctxq:
                        ctxq.pop(0)()
                    if pending and h >= 2:
                        pending.pop(0)()
                    ctxq = mk_ctx_chunks(j, h, pts)
                while ctxq:
                    ctxq.pop(0)()
                while main:
                    main.pop(0)()
                # lookahead: S + exp for the first heads of block j+1 so the
                # scalar engine never starves at block boundaries (block 3 is
                # exp-bound; its S work can start as soon as proj(j+1) lands)
                def rfiller(k):
                    if reserve:
                        reserve.pop(0)()
                for h2 in look:
                    pts_store[(j + 1, h2)] = s_phase(j + 1, h2, rfiller)
                while reserve:
                    reserve.pop(0)()
                while pending:
                    pending.pop(0)()

            xq_t, xb_t = xq_0, xb_0
            for g in project_groups(0, xq_t, xb_t):
                g()
            LOOK = [[0], [0, 1], [0, 1], []]
            for tb in range(NTB):
                ilq = []
                if tb + 1 < NTB:
                    xq_t, xb_t = load_x(tb + 1)
                    ilq += project_groups(tb + 1, xq_t, xb_t)
                if tb >= 1:
                    ilq += output_groups(tb - 1)
                attend(tb, ilq, LOOK[tb])
            for g in output_groups(NTB - 1):
                g()

    nc.compile()
    return nc


def make_in_maps(x, Wq, Wk, Wv, Wo):
    import ml_dtypes

    f8 = ml_dtypes.float8_e4m3
    b16 = ml_dtypes.bfloat16
    x = np.asarray(x, np.float32)
    Wq, Wk, Wv, Wo = (np.asarray(w, np.float32) for w in (Wq, Wk, Wv, Wo))

    def dr_layout(a):
        # [C, N] -> DoubleRow fp8 [C/2, 2*N]: channel ch = 256*dc + 128*t + p
        Cd, N = a.shape
        return np.ascontiguousarray(
            a.reshape(Cd // 256, 2, 128, N).transpose(0, 2, 1, 3)
            .reshape(Cd // 2, 2 * N).astype(f8))

    tri = (np.arange(128)[None, :] >= np.arange(128)[:, None]).astype(b16)
    zero8 = np.zeros((128, T_FULL), f8)

    in_maps = []
    for core in range(NCORES):
        b, g = divmod(core, GROUPS)
        sl = slice(g * OC, (g + 1) * OC)
        xT = np.ascontiguousarray(x[b].T)
        in_maps.append({
            "xq8": dr_layout(xT),
            "xb16": xT.astype(b16),
            "wq8": dr_layout(np.ascontiguousarray(Wq[sl, :].T) * WSCALE),
            "wk8": dr_layout(np.ascontiguousarray(Wk[sl, :].T) * WSCALE),
            "wv16": np.ascontiguousarray(Wv[sl, :].T).astype(b16),
            "wo16": np.ascontiguousarray(Wo[:, sl].T).astype(b16),
            "tri": tri,
            "zero8": zero8,
        })
    return in_maps


def _run(inputs, trace=False):
    from concourse.bass_utils import run_bass_kernel_spmd

    nc = build_program()
    in_maps = make_in_maps(
        inputs["x"], inputs["Wq"], inputs["Wk"], inputs["Wv"], inputs["Wo"])
    res = run_bass_kernel_spmd(nc, in_maps, core_ids=list(range(NCORES)), trace=trace)
    y = np.zeros((B, T_FULL, C), np.float32)
    for core in range(NCORES):
        y[core // GROUPS] += res.results[core]["yT"].astype(np.float32).T
    return y, res


def kernel(**inputs):
    y, _ = _run(inputs)
    return y


# revision 25
# speedup vs baseline: 1.8033x; 1.0084x over previous
"""Causal multi-head attention (B=4, T=2048, C=1024, H=16, HD=64) on 8 TRN2
NeuronCores.

Sharding: 2D - batch (4) x head-group (2 groups of 8 heads). Each core handles
one batch's tokens for 8 heads (OC = 512 local channels); host sums the two
group partials of y.

v2 layout/precision strategy:
  - Q/K projections run in fp8e4 DoubleRow mode (x and 32*Wq/32*Wk quantized
    to fp8 on host, contraction 256/instr at 0.5 cyc/row). The 32*32=1024
    factor is folded into the exp scale.
  - Q^T/K^T stored as fp8 [128, 2, T] DoubleRow operands (t=1 plane zeroed,
    qt additionally zero on the other head's 64 rows); S^T chunks
    [k=128, q<=512] via fp8 DoubleRow at 0.5 cyc/row.
  - V projection and output projection in bf16 (1 cyc/row).
  - Causal q-range restriction at 128-granularity on S, exp, and ctx
    (diagonal chunks only compute q >= 128*kc).
  - Masking: DVE multiplies with host-provided lower-tri mask tiles (T0/T1)
    on the two diagonal chunks per (head, block) - gpsimd only does the
    denominator partition_broadcast and v-ones memsets.
  - ctx matmuls bf16 with the [V_h | 1] ones-column denominator trick;
    normalize via broadcast + DVE reciprocal + fused mul into bf16 ct.
  - y^T computed in bf16, host converts/sums in f32.
"""

import numpy as np

B, T_FULL, C = 4, 2048, 1024
H, HD = 16, 64
GROUPS = 2
HL = H // GROUPS          # heads per core = 8
OC = HL * HD              # local channels = 512
P = 128                   # partitions
TB = 512                  # token block (moving dim)
SCALE = float(1.0 / np.sqrt(HD))
NCORES = 8
WSCALE = 32.0             # host premultiplies Wq/Wk before fp8 quantization
VSTRIDE = 72              # per-head column stride in v tiles (64 d + 1 one + 7 pad)


def build_program(T=T_FULL):
    from contextlib import ExitStack

    import concourse.bacc as bacc
    import concourse.mybir as mybir
    import concourse.tile as tile

    f32 = mybir.dt.float32
    bf16 = mybir.dt.bfloat16
    fp8 = mybir.dt.float8e4
    EXP = mybir.ActivationFunctionType.Exp
    DR = mybir.MatmulPerfMode.DoubleRow
    ESCALE = SCALE / (WSCALE * WSCALE)

    NTB = T // TB             # 512-token blocks
    NKC = T // P              # 128-token key chunks
    CCH = C // P              # 8 contraction chunks of C
    DCH = C // (2 * P)        # 4 double-contraction chunks (fp8 DoubleRow)
    MCH = OC // P             # 4 output-channel chunks

    nc = bacc.Bacc("TRN2", target_bir_lowering=False, debug=False)
    xq8d = nc.dram_tensor("xq8", [C // 2, 2 * T], fp8, kind="ExternalInput").ap()
    xb16d = nc.dram_tensor("xb16", [C, T], bf16, kind="ExternalInput").ap()
    wq8d = nc.dram_tensor("wq8", [C // 2, 2 * OC], fp8, kind="ExternalInput").ap()
    wk8d = nc.dram_tensor("wk8", [C // 2, 2 * OC], fp8, kind="ExternalInput").ap()
    wv16d = nc.dram_tensor("wv16", [C, OC], bf16, kind="ExternalInput").ap()
    wo16d = nc.dram_tensor("wo16", [OC, C], bf16, kind="ExternalInput").ap()
    trid = nc.dram_tensor("tri", [P, P], bf16, kind="ExternalInput").ap()
    zerod = nc.dram_tensor("zero8", [P, T], fp8, kind="ExternalInput").ap()
    yT = nc.dram_tensor("yT", [C, T], bf16, kind="ExternalOutput").ap()

    with tile.TileContext(nc) as tc, ExitStack() as ctx:
        perm = ctx.enter_context(tc.tile_pool(name="perm", bufs=1))
        # per-head Q^T fp8 [128, 2, T]: rows (h%2)*64..+64 of t=0 hold the
        # head's channels; everything else stays zero (zero-DMA'd once) so the
        # DoubleRow contraction only sees this head's 64 channels.
        qt = [perm.tile([P, 2 * T], fp8, tag=f"qt{h}", name=f"qt{h}")
              for h in range(HL)]
        # per-pair K^T fp8 [128, 2, T]: t=0 holds both heads' channels
        # (rows = oc chunk), t=1 zero.
        kt = [perm.tile([P, 2 * T], fp8, tag=f"kt{m}", name=f"kt{m}")
              for m in range(MCH)]
        # normalized ctx^T bf16 [oc rows, T] per m-chunk
        ct = [perm.tile([P, T], bf16, tag=f"ct{m}", name=f"ct{m}") for m in range(MCH)]
        # V bf16 per key chunk: head stride 72 = [64 vals | 1 | 7 junk]
        v = [perm.tile([P, HL * VSTRIDE], bf16, tag=f"v{t}", name=f"v{t}")
             for t in range(NKC)]
        tri = perm.tile([P, P], bf16, tag="tri", name="tri")
        ONE_BF16 = 0x3F80
        for vt in v:
            vv = vt.rearrange("p (h e) -> p h e", e=VSTRIDE)
            nc.gpsimd.memset(vv[:, :, 64:65].bitcast(mybir.dt.uint16), ONE_BF16)

        with (
            tc.tile_pool(name="wpool", bufs=1) as wp,
            tc.tile_pool(name="xqpool", bufs=2) as xqp,
            tc.tile_pool(name="xbpool", bufs=2) as xbp,
            tc.tile_pool(name="ptpool", bufs=32) as ptp,
            tc.tile_pool(name="tmppool", bufs=2) as tmp,
            tc.tile_pool(name="ypool", bufs=2) as yp,
            tc.tile_pool(name="mmps", bufs=2, space="PSUM") as pp,
            tc.tile_pool(name="stps", bufs=2, space="PSUM") as stp,
            tc.tile_pool(name="ctxps", bufs=2, space="PSUM") as cxp,
        ):
            xq8dv = xq8d.rearrange("p (t f) -> p t f", t=2)

            def load_x(tb):
                xq_t, xb_t = [], []
                for d in range(DCH):
                    t_ = xqp.tile([P, 2 * TB], fp8, tag=f"xq{d}",
                                  name=f"xq_{tb}_{d}")
                    nc.sync.dma_start(
                        out=t_.rearrange("p (t f) -> p t f", t=2),
                        in_=xq8dv[d * P:(d + 1) * P, :, tb * TB:(tb + 1) * TB])
                    xq_t.append(t_)
                for c in range(CCH):
                    t_ = xbp.tile([P, TB], bf16, tag=f"xb{c}",
                                  name=f"xb_{tb}_{c}")
                    nc.sync.dma_start(
                        out=t_, in_=xb16d[c * P:(c + 1) * P,
                                          tb * TB:(tb + 1) * TB])
                    xb_t.append(t_)
                return xq_t, xb_t

            # ---- loads, ordered so block-0's dependency chain lands first:
            # tri; x/wq8/wk8 block-0; per-tile zero planes (gate the first
            # casts / S matmuls); xb16/wv block-0; wo streams in behind.
            nc.sync.dma_start(out=tri, in_=trid)
            wq, wk, wv, wo = [], [], [], []
            xq_0 = []
            for d in range(DCH):
                t_ = xqp.tile([P, 2 * TB], fp8, tag=f"xq{d}", name=f"xq_0_{d}")
                nc.sync.dma_start(
                    out=t_.rearrange("p (t f) -> p t f", t=2),
                    in_=xq8dv[d * P:(d + 1) * P, :, 0:TB])
                xq_0.append(t_)
                t_ = wp.tile([P, 2 * OC], fp8, tag=f"wq{d}", name=f"wq{d}")
                nc.sync.dma_start(out=t_, in_=wq8d[d * P:(d + 1) * P, :])
                wq.append(t_)
            for d in range(DCH):
                t_ = wp.tile([P, 2 * OC], fp8, tag=f"wk{d}", name=f"wk{d}")
                nc.sync.dma_start(out=t_, in_=wk8d[d * P:(d + 1) * P, :])
                wk.append(t_)
            # qt: zero the other head's rows (t=0) and the whole t=1 plane;
            # kt: zero only the t=1 plane (t=0 is fully cast-written).
            for h in range(HL):
                z0 = (1 - h % 2) * 64
                nc.sync.dma_start(out=qt[h][z0:z0 + 64, 0:T],
                                  in_=zerod[z0:z0 + 64, 0:T])
                nc.sync.dma_start(out=qt[h][:, T:2 * T], in_=zerod[:, 0:T])
                if h % 2 == 1:
                    nc.sync.dma_start(out=kt[h // 2][:, T:2 * T],
                                      in_=zerod[:, 0:T])
            xb_0 = []
            for c in range(CCH):
                t_ = xbp.tile([P, TB], bf16, tag=f"xb{c}", name=f"xb_0_{c}")
                nc.sync.dma_start(out=t_, in_=xb16d[c * P:(c + 1) * P, 0:TB])
                xb_0.append(t_)
                t_ = wp.tile([P, OC], bf16, tag=f"wv{c}", name=f"wv{c}")
                nc.sync.dma_start(out=t_, in_=wv16d[c * P:(c + 1) * P, :])
                wv.append(t_)
            for ci in range(MCH):
                t_ = wp.tile([P, C], bf16, tag=f"wo{ci}", name=f"wo{ci}")
                nc.sync.dma_start(out=t_, in_=wo16d[ci * P:(ci + 1) * P, :])
                wo.append(t_)

            def project_groups(tb, xq_t, xb_t):
                groups = []

                def proj_qk(w, isq, m, tb=tb):
                    def go():
                        ps = pp.tile([P, TB], f32, tag="mm512",
                                     name=f"ps_{tb}_{m}_{isq}")
                        for d in range(DCH):
                            nc.tensor.matmul(
                                ps,
                                lhsT=w[d].rearrange("p (t o) -> p t o", t=2)[
                                    :, :, m * P:(m + 1) * P],
                                rhs=xq_t[d].rearrange("p (t f) -> p t f", t=2),
                                start=(d == 0), stop=(d == DCH - 1),
                                perf_mode=DR)
                        if isq:
                            # write each head's rows into its own qt tile
                            for hh in (0, 1):
                                r0_ = hh * 64
                                nc.vector.tensor_copy(
                                    qt[2 * m + hh][r0_:r0_ + 64,
                                                   tb * TB:(tb + 1) * TB],
                                    ps[r0_:r0_ + 64, :])
                        else:
                            nc.vector.tensor_copy(
                                kt[m][:, tb * TB:(tb + 1) * TB], ps)
                    return go

                def proj_v(ts_, tb=tb):
                    def go():
                        ps = pp.tile([P, OC], f32, tag="mm512",
                                     name=f"psv_{tb}_{ts_}")
                        for c in range(CCH):
                            nc.tensor.matmul(
                                ps,
                                lhsT=xb_t[c][:, ts_ * P:(ts_ + 1) * P],
                                rhs=wv[c],
                                start=(c == 0), stop=(c == CCH - 1))
                        ti = tb * (TB // P) + ts_
                        nc.vector.tensor_copy(
                            v[ti].rearrange("p (h e) -> p h e", e=VSTRIDE)[:, :, 0:64],
                            ps.rearrange("p (h d) -> p h d", d=64))
                    return go

                # zip Q and K per m-chunk so early heads' S deps land first
                for m in range(MCH):
                    groups.append(proj_qk(wq, True, m))
                    groups.append(proj_qk(wk, False, m))
                for ts_ in range(TB // P):
                    groups.append(proj_v(ts_))
                return groups

            def output_groups(tb):
                def out_co(co, tb=tb):
                    def go():
                        ps = pp.tile([P, TB], f32, tag="mm512",
                                     name=f"yps_{co}_{tb}")
                        for ci in range(MCH):
                            nc.tensor.matmul(
                                ps, lhsT=wo[ci][:, co * P:(co + 1) * P],
                                rhs=ct[ci][:, tb * TB:(tb + 1) * TB],
                                start=(ci == 0), stop=(ci == MCH - 1))
                        ysb = yp.tile([P, TB], bf16, tag="ysb", name=f"ysb_{co}_{tb}")
                        nc.vector.tensor_copy(ysb, ps)
                        nc.sync.dma_start(
                            out=yT[co * P:(co + 1) * P, tb * TB:(tb + 1) * TB],
                            in_=ysb)
                    return go
                return [out_co(co) for co in range(C // P)]

            pending = []

            def mk_norm(h, j, m, r0, ctx_ps):
                def norm():
                    s_sb = tmp.tile([1, TB], f32, tag="s", bufs=1, name=f"s_{h}_{j}")
                    nc.vector.tensor_copy(s_sb, ctx_ps[64:65, :])
                    rb = tmp.tile([64, TB], f32, tag="rb", bufs=1, name=f"rb_{h}_{j}")
                    nc.gpsimd.partition_broadcast(rb, s_sb)
                    nc.vector.reciprocal_approx_fast(out=rb, in_=rb)
                    nc.vector.tensor_mul(
                        ct[m][r0:r0 + 64, j * TB:(j + 1) * TB], ctx_ps[0:64, :], rb)
                return norm

            pts_store = {}

            def s_phase(j, h, filler):
                # S + exp + mask for head (j, h), diagonal pairs first (their
                # longer exp->mask chain overlaps later S matmuls); filler(k)
                # is called between pairs to keep the PE busy while S stalls
                # on the st pool (exp rate-bound).
                npair = 2 * (j + 1)
                m = h // 2
                ktv = kt[m].rearrange("p (t f) -> p t f", t=2)
                qtv = qt[h].rearrange("p (t f) -> p t f", t=2)
                pts = [None] * npair
                for pp_ in range(npair - 1, -1, -1):
                    st = stp.tile([P, 2 * TB], f32, tag="st",
                                  name=f"st_{h}_{j}_{pp_}")
                    qlo_pair = max(0, 128 * 2 * pp_ - TB * j)
                    for t in (0, 1):
                        c = 2 * pp_ + t
                        qlo = max(0, 128 * c - TB * j)
                        nc.tensor.matmul(
                            st[:, t * TB + qlo:(t + 1) * TB],
                            lhsT=ktv[:, :, c * P:(c + 1) * P],
                            rhs=qtv[:, :, j * TB + qlo:(j + 1) * TB],
                            start=True, stop=True, skip_group_check=True,
                            perf_mode=DR)
                    pt_ = ptp.tile([P, 2 * TB], bf16, tag="pt",
                                   name=f"pt_{h}_{j}_{pp_}")
                    pt3 = pt_.rearrange("p (t f) -> p t f", t=2)
                    st3 = st.rearrange("p (t f) -> p t f", t=2)
                    nc.scalar.activation(
                        pt3[:, :, qlo_pair:], st3[:, :, qlo_pair:],
                        EXP, scale=ESCALE)
                    if 2 * pp_ + 1 >= 4 * j:
                        # diagonal pair: tri-mask each chunk's 128-wide
                        # diagonal square (on DVE - the gpsimd queue is
                        # in-order and slow, serializing this chain)
                        for t in (0, 1):
                            off = t * TB + 128 * (2 * pp_ + t) - TB * j
                            nc.vector.tensor_mul(
                                pt_[:, off:off + 128],
                                pt_[:, off:off + 128], tri)
                    pts[pp_] = pt_
                    filler(2)
                return pts

            def mk_ctx_chunks(j, h, pts):
                # ctx matmuls for head h as small closures, ascending pair
                # order (pair 0's chunk 0 covers the full q range so its
                # start=True zeroes the whole psum). The psum tile is
                # allocated lazily by the first closure.
                nch = 4 * (j + 1)
                box = [None]
                out = []

                def pair_go(pp_):
                    def go():
                        if box[0] is None:
                            box[0] = cxp.tile([P, TB], f32, tag="ctx",
                                              name=f"cps_{h}_{j}")
                        for t in (0, 1):
                            c = 2 * pp_ + t
                            qlo = max(0, 128 * c - TB * j)
                            nmm = 2 * pp_ + t
                            nc.tensor.matmul(
                                box[0][0:65, qlo:TB],
                                lhsT=v[c][:, h * VSTRIDE:h * VSTRIDE + 65],
                                rhs=pts[pp_][:, t * TB + qlo:(t + 1) * TB],
                                start=(nmm == 0), stop=(nmm == nch - 1),
                                skip_group_check=True)
                    return go

                for pp_ in range(nch // 2):
                    out.append(pair_go(pp_))
                out.append(lambda: pending.append(
                    mk_norm(h, j, h // 2, (h % 2) * 64, box[0])))
                return out

            def attend(j, ilq, look):
                reserve = ilq[-2:]
                main = ilq[:max(0, len(ilq) - 2)]
                ctxq = []

                def filler(k):
                    for _ in range(k):
                        if ctxq:
                            ctxq.pop(0)()
                        elif main:
                            main.pop(0)()
                            return

                for h in range(HL):
                    pts = pts_store.pop((j, h), None)
                    if pts is None:
                        pts = s_phase(j, h, filler)
                    else:
                        filler(3)
                    while ctxq:
                        ctxq.pop(0)()
                    if pending and h >= 2:
                        pending.pop(0)()
                    ctxq = mk_ctx_chunks(j, h, pts)
                while ctxq:
                    ctxq.pop(0)()
                while main:
                    main.pop(0)()
                # lookahead: S + exp for the first heads of block j+1 so the
                # scalar engine never starves at block boundaries (block 3 is
                # exp-bound; its S work can start as soon as proj(j+1) lands)
                def rfiller(k):
                    if reserve:
                        reserve.pop(0)()
                for h2 in look:
                    pts_store[(j + 1, h2)] = s_phase(j + 1, h2, rfiller)
                while reserve:
                    reserve.pop(0)()
                while pending:
                    pending.pop(0)()

            xq_t, xb_t = xq_0, xb_0
            for g in project_groups(0, xq_t, xb_t):
                g()
            LOOK = [[0, 1], [0, 1], [0, 1], []]
            for tb in range(NTB):
                ilq = []
                if tb + 1 < NTB:
                    xq_t, xb_t = load_x(tb + 1)
                    ilq += project_groups(tb + 1, xq_t, xb_t)
                if tb >= 1:
                    ilq += output_groups(tb - 1)
                attend(tb, ilq, LOOK[tb])
            for g in output_groups(NTB - 1):
                g()

    nc.compile()
    return nc


def make_in_maps(x, Wq, Wk, Wv, Wo):
    import ml_dtypes

    f8 = ml_dtypes.float8_e4m3
    b16 = ml_dtypes.bfloat16
    x = np.asarray(x, np.float32)
    Wq, Wk, Wv, Wo = (np.asarray(w, np.float32) for w in (Wq, Wk, Wv, Wo))

    def dr_layout(a):
        # [C, N] -> DoubleRow fp8 [C/2, 2*N]: channel ch = 256*dc + 128*t + p
        Cd, N = a.shape
        return np.ascontiguousarray(
            a.reshape(Cd // 256, 2, 128, N).transpose(0, 2, 1, 3)
            .reshape(Cd // 2, 2 * N).astype(f8))

    tri = (np.arange(128)[None, :] >= np.arange(128)[:, None]).astype(b16)
    zero8 = np.zeros((128, T_FULL), f8)

    in_maps = []
    for core in range(NCORES):
        b, g = divmod(core, GROUPS)
        sl = slice(g * OC, (g + 1) * OC)
        xT = np.ascontiguousarray(x[b].T)
        in_maps.append({
            "xq8": dr_layout(xT),
            "xb16": xT.astype(b16),
            "wq8": dr_layout(np.ascontiguousarray(Wq[sl, :].T) * WSCALE),
            "wk8": dr_layout(np.ascontiguousarray(Wk[sl, :].T) * WSCALE),
            "wv16": np.ascontiguousarray(Wv[sl, :].T).astype(b16),
            "wo16": np.ascontiguousarray(Wo[:, sl].T).astype(b16),
            "tri": tri,
            "zero8": zero8,
        })
    return in_maps


def _run(inputs, trace=False):
    from concourse.bass_utils import run_bass_kernel_spmd

    nc = build_program()
    in_maps = make_in_maps(
        inputs["x"], inputs["Wq"], inputs["Wk"], inputs["Wv"], inputs["Wo"])
    res = run_bass_kernel_spmd(nc, in_maps, core_ids=list(range(NCORES)), trace=trace)
    y = np.zeros((B, T_FULL, C), np.float32)
    for core in range(NCORES):
        y[core // GROUPS] += res.results[core]["yT"].astype(np.float32).T
    return y, res


def kernel(**inputs):
    y, _ = _run(inputs)
    return y


# revision 26
# speedup vs baseline: 1.8288x; 1.0141x over previous
"""Causal multi-head attention (B=4, T=2048, C=1024, H=16, HD=64) on 8 TRN2
NeuronCores.

Sharding: 2D - batch (4) x head-group (2 groups of 8 heads). Each core handles
one batch's tokens for 8 heads (OC = 512 local channels); host sums the two
group partials of y.

v2 layout/precision strategy:
  - Q/K projections run in fp8e4 DoubleRow mode (x and 32*Wq/32*Wk quantized
    to fp8 on host, contraction 256/instr at 0.5 cyc/row). The 32*32=1024
    factor is folded into the exp scale.
  - Q^T/K^T stored as fp8 [128, 2, T] DoubleRow operands (t=1 plane zeroed,
    qt additionally zero on the other head's 64 rows); S^T chunks
    [k=128, q<=512] via fp8 DoubleRow at 0.5 cyc/row.
  - V projection and output projection in bf16 (1 cyc/row).
  - Causal q-range restriction at 128-granularity on S, exp, and ctx
    (diagonal chunks only compute q >= 128*kc).
  - Masking: DVE multiplies with host-provided lower-tri mask tiles (T0/T1)
    on the two diagonal chunks per (head, block) - gpsimd only does the
    denominator partition_broadcast and v-ones memsets.
  - ctx matmuls bf16 with the [V_h | 1] ones-column denominator trick;
    normalize via broadcast + DVE reciprocal + fused mul into bf16 ct.
  - y^T computed in bf16, host converts/sums in f32.
"""

import numpy as np

B, T_FULL, C = 4, 2048, 1024
H, HD = 16, 64
GROUPS = 2
HL = H // GROUPS          # heads per core = 8
OC = HL * HD              # local channels = 512
P = 128                   # partitions
TB = 512                  # token block (moving dim)
SCALE = float(1.0 / np.sqrt(HD))
NCORES = 8
WSCALE = 32.0             # host premultiplies Wq/Wk before fp8 quantization
VSTRIDE = 72              # per-head column stride in v tiles (64 d + 1 one + 7 pad)


def build_program(T=T_FULL):
    from contextlib import ExitStack

    import concourse.bacc as bacc
    import concourse.mybir as mybir
    import concourse.tile as tile

    f32 = mybir.dt.float32
    bf16 = mybir.dt.bfloat16
    fp8 = mybir.dt.float8e4
    EXP = mybir.ActivationFunctionType.Exp
    DR = mybir.MatmulPerfMode.DoubleRow
    ESCALE = SCALE / (WSCALE * WSCALE)

    NTB = T // TB             # 512-token blocks
    NKC = T // P              # 128-token key chunks
    CCH = C // P              # 8 contraction chunks of C
    DCH = C // (2 * P)        # 4 double-contraction chunks (fp8 DoubleRow)
    MCH = OC // P             # 4 output-channel chunks

    nc = bacc.Bacc("TRN2", target_bir_lowering=False, debug=False)
    xq8d = nc.dram_tensor("xq8", [C // 2, 2 * T], fp8, kind="ExternalInput").ap()
    xb16d = nc.dram_tensor("xb16", [C, T], bf16, kind="ExternalInput").ap()
    wq8d = nc.dram_tensor("wq8", [C // 2, 2 * OC], fp8, kind="ExternalInput").ap()
    wk8d = nc.dram_tensor("wk8", [C // 2, 2 * OC], fp8, kind="ExternalInput").ap()
    wv16d = nc.dram_tensor("wv16", [C, OC], bf16, kind="ExternalInput").ap()
    wo16d = nc.dram_tensor("wo16", [OC, C], bf16, kind="ExternalInput").ap()
    trid = nc.dram_tensor("tri", [P, P], bf16, kind="ExternalInput").ap()
    zerod = nc.dram_tensor("zero8", [P, T], fp8, kind="ExternalInput").ap()
    yT = nc.dram_tensor("yT", [C, T], bf16, kind="ExternalOutput").ap()

    with tile.TileContext(nc) as tc, ExitStack() as ctx:
        perm = ctx.enter_context(tc.tile_pool(name="perm", bufs=1))
        # per-head Q^T fp8 [128, 2, T]: rows (h%2)*64..+64 of t=0 hold the
        # head's channels; everything else stays zero (zero-DMA'd once) so the
        # DoubleRow contraction only sees this head's 64 channels.
        qt = [perm.tile([P, 2 * T], fp8, tag=f"qt{h}", name=f"qt{h}")
              for h in range(HL)]
        # per-pair K^T fp8 [128, 2, T]: t=0 holds both heads' channels
        # (rows = oc chunk), t=1 zero.
        kt = [perm.tile([P, 2 * T], fp8, tag=f"kt{m}", name=f"kt{m}")
              for m in range(MCH)]
        # normalized ctx^T bf16 [oc rows, T] per m-chunk
        ct = [perm.tile([P, T], bf16, tag=f"ct{m}", name=f"ct{m}") for m in range(MCH)]
        # V bf16 per key chunk: head stride 72 = [64 vals | 1 | 7 junk]
        v = [perm.tile([P, HL * VSTRIDE], bf16, tag=f"v{t}", name=f"v{t}")
             for t in range(NKC)]
        tri = perm.tile([P, P], bf16, tag="tri", name="tri")
        ONE_BF16 = 0x3F80
        for vt in v:
            vv = vt.rearrange("p (h e) -> p h e", e=VSTRIDE)
            nc.gpsimd.memset(vv[:, :, 64:65].bitcast(mybir.dt.uint16), ONE_BF16)

        with (
            tc.tile_pool(name="wpool", bufs=1) as wp,
            tc.tile_pool(name="xqpool", bufs=2) as xqp,
            tc.tile_pool(name="xbpool", bufs=2) as xbp,
            tc.tile_pool(name="ptpool", bufs=32) as ptp,
            tc.tile_pool(name="tmppool", bufs=2) as tmp,
            tc.tile_pool(name="ypool", bufs=2) as yp,
            tc.tile_pool(name="mmps", bufs=2, space="PSUM") as pp,
            tc.tile_pool(name="stps", bufs=2, space="PSUM") as stp,
            tc.tile_pool(name="ctxps", bufs=2, space="PSUM") as cxp,
        ):
            xq8dv = xq8d.rearrange("p (t f) -> p t f", t=2)

            def load_x(tb):
                xq_t, xb_t = [], []
                for d in range(DCH):
                    t_ = xqp.tile([P, 2 * TB], fp8, tag=f"xq{d}",
                                  name=f"xq_{tb}_{d}")
                    nc.sync.dma_start(
                        out=t_.rearrange("p (t f) -> p t f", t=2),
                        in_=xq8dv[d * P:(d + 1) * P, :, tb * TB:(tb + 1) * TB])
                    xq_t.append(t_)
                for c in range(CCH):
                    t_ = xbp.tile([P, TB], bf16, tag=f"xb{c}",
                                  name=f"xb_{tb}_{c}")
                    nc.sync.dma_start(
                        out=t_, in_=xb16d[c * P:(c + 1) * P,
                                          tb * TB:(tb + 1) * TB])
                    xb_t.append(t_)
                return xq_t, xb_t

            # ---- loads, ordered so block-0's dependency chain lands first:
            # tri; x/wq8/wk8 block-0; per-tile zero planes (gate the first
            # casts / S matmuls); xb16/wv block-0; wo streams in behind.
            nc.sync.dma_start(out=tri, in_=trid)
            wq, wk, wv, wo = [], [], [], []
            xq_0 = []
            for d in range(DCH):
                t_ = xqp.tile([P, 2 * TB], fp8, tag=f"xq{d}", name=f"xq_0_{d}")
                nc.sync.dma_start(
                    out=t_.rearrange("p (t f) -> p t f", t=2),
                    in_=xq8dv[d * P:(d + 1) * P, :, 0:TB])
                xq_0.append(t_)
                t_ = wp.tile([P, 2 * OC], fp8, tag=f"wq{d}", name=f"wq{d}")
                nc.sync.dma_start(out=t_, in_=wq8d[d * P:(d + 1) * P, :])
                wq.append(t_)
            for d in range(DCH):
                t_ = wp.tile([P, 2 * OC], fp8, tag=f"wk{d}", name=f"wk{d}")
                nc.sync.dma_start(out=t_, in_=wk8d[d * P:(d + 1) * P, :])
                wk.append(t_)
            # qt: zero the other head's rows (t=0) and the whole t=1 plane;
            # kt: zero only the t=1 plane (t=0 is fully cast-written).
            for h in range(HL):
                z0 = (1 - h % 2) * 64
                nc.sync.dma_start(out=qt[h][z0:z0 + 64, 0:T],
                                  in_=zerod[z0:z0 + 64, 0:T])
                nc.sync.dma_start(out=qt[h][:, T:2 * T], in_=zerod[:, 0:T])
                if h % 2 == 1:
                    nc.sync.dma_start(out=kt[h // 2][:, T:2 * T],
                                      in_=zerod[:, 0:T])
            xb_0 = []
            for c in range(CCH):
                t_ = xbp.tile([P, TB], bf16, tag=f"xb{c}", name=f"xb_0_{c}")
                nc.sync.dma_start(out=t_, in_=xb16d[c * P:(c + 1) * P, 0:TB])
                xb_0.append(t_)
                t_ = wp.tile([P, OC], bf16, tag=f"wv{c}", name=f"wv{c}")
                nc.sync.dma_start(out=t_, in_=wv16d[c * P:(c + 1) * P, :])
                wv.append(t_)
            for ci in range(MCH):
                t_ = wp.tile([P, C], bf16, tag=f"wo{ci}", name=f"wo{ci}")
                nc.sync.dma_start(out=t_, in_=wo16d[ci * P:(ci + 1) * P, :])
                wo.append(t_)

            def project_groups(tb, xq_t, xb_t):
                groups = []

                def proj_qk(w, isq, m, tb=tb):
                    def go():
                        ps = pp.tile([P, TB], f32, tag="mm512",
                                     name=f"ps_{tb}_{m}_{isq}")
                        for d in range(DCH):
                            nc.tensor.matmul(
                                ps,
                                lhsT=w[d].rearrange("p (t o) -> p t o", t=2)[
                                    :, :, m * P:(m + 1) * P],
                                rhs=xq_t[d].rearrange("p (t f) -> p t f", t=2),
                                start=(d == 0), stop=(d == DCH - 1),
                                perf_mode=DR)
                        if isq:
                            # write each head's rows into its own qt tile
                            for hh in (0, 1):
                                r0_ = hh * 64
                                nc.vector.tensor_copy(
                                    qt[2 * m + hh][r0_:r0_ + 64,
                                                   tb * TB:(tb + 1) * TB],
                                    ps[r0_:r0_ + 64, :])
                        else:
                            nc.vector.tensor_copy(
                                kt[m][:, tb * TB:(tb + 1) * TB], ps)
                    return go

                def proj_v(ts_, tb=tb):
                    def go():
                        ps = pp.tile([P, OC], f32, tag="mm512",
                                     name=f"psv_{tb}_{ts_}")
                        for c in range(CCH):
                            nc.tensor.matmul(
                                ps,
                                lhsT=xb_t[c][:, ts_ * P:(ts_ + 1) * P],
                                rhs=wv[c],
                                start=(c == 0), stop=(c == CCH - 1))
                        ti = tb * (TB // P) + ts_
                        nc.vector.tensor_copy(
                            v[ti].rearrange("p (h e) -> p h e", e=VSTRIDE)[:, :, 0:64],
                            ps.rearrange("p (h d) -> p h d", d=64))
                    return go

                # zip Q and K per m-chunk so early heads' S deps land first
                for m in range(MCH):
                    groups.append(proj_qk(wq, True, m))
                    groups.append(proj_qk(wk, False, m))
                for ts_ in range(TB // P):
                    groups.append(proj_v(ts_))
                return groups

            def output_groups(tb):
                def out_co(co, tb=tb):
                    def go():
                        ps = pp.tile([P, TB], f32, tag="mm512",
                                     name=f"yps_{co}_{tb}")
                        for ci in range(MCH):
                            nc.tensor.matmul(
                                ps, lhsT=wo[ci][:, co * P:(co + 1) * P],
                                rhs=ct[ci][:, tb * TB:(tb + 1) * TB],
                                start=(ci == 0), stop=(ci == MCH - 1))
                        ysb = yp.tile([P, TB], bf16, tag="ysb", name=f"ysb_{co}_{tb}")
                        nc.vector.tensor_copy(ysb, ps)
                        nc.sync.dma_start(
                            out=yT[co * P:(co + 1) * P, tb * TB:(tb + 1) * TB],
                            in_=ysb)
                    return go
                return [out_co(co) for co in range(C // P)]

            pending = []

            def mk_norm(h, j, m, r0, ctx_ps):
                def norm():
                    s_sb = tmp.tile([1, TB], f32, tag="s", bufs=1, name=f"s_{h}_{j}")
                    nc.vector.tensor_copy(s_sb, ctx_ps[64:65, :])
                    rb = tmp.tile([64, TB], f32, tag="rb", bufs=1, name=f"rb_{h}_{j}")
                    nc.gpsimd.partition_broadcast(rb, s_sb)
                    nc.vector.reciprocal_approx_fast(out=rb, in_=rb)
                    nc.vector.tensor_mul(
                        ct[m][r0:r0 + 64, j * TB:(j + 1) * TB], ctx_ps[0:64, :], rb)
                return norm

            pts_store = {}

            def s_phase(j, h, filler):
                # S + exp + mask for head (j, h), diagonal pairs first (their
                # longer exp->mask chain overlaps later S matmuls); filler(k)
                # is called between pairs to keep the PE busy while S stalls
                # on the st pool (exp rate-bound).
                npair = 2 * (j + 1)
                m = h // 2
                ktv = kt[m].rearrange("p (t f) -> p t f", t=2)
                qtv = qt[h].rearrange("p (t f) -> p t f", t=2)
                pts = [None] * npair
                for pp_ in range(npair - 1, -1, -1):
                    st = stp.tile([P, 2 * TB], f32, tag="st",
                                  name=f"st_{h}_{j}_{pp_}")
                    qlo_pair = max(0, 128 * 2 * pp_ - TB * j)
                    for t in (0, 1):
                        c = 2 * pp_ + t
                        qlo = max(0, 128 * c - TB * j)
                        nc.tensor.matmul(
                            st[:, t * TB + qlo:(t + 1) * TB],
                            lhsT=ktv[:, :, c * P:(c + 1) * P],
                            rhs=qtv[:, :, j * TB + qlo:(j + 1) * TB],
                            start=True, stop=True, skip_group_check=True,
                            perf_mode=DR)
                    pt_ = ptp.tile([P, 2 * TB], bf16, tag="pt",
                                   name=f"pt_{h}_{j}_{pp_}")
                    pt3 = pt_.rearrange("p (t f) -> p t f", t=2)
                    st3 = st.rearrange("p (t f) -> p t f", t=2)
                    nc.scalar.activation(
                        pt3[:, :, qlo_pair:], st3[:, :, qlo_pair:],
                        EXP, scale=ESCALE)
                    if 2 * pp_ + 1 >= 4 * j:
                        # diagonal pair: tri-mask each chunk's 128-wide
                        # diagonal square (on DVE - the gpsimd queue is
                        # in-order and slow, serializing this chain)
                        for t in (0, 1):
                            off = t * TB + 128 * (2 * pp_ + t) - TB * j
                            nc.vector.tensor_mul(
                                pt_[:, off:off + 128],
                                pt_[:, off:off + 128], tri)
                    pts[pp_] = pt_
                    filler(2)
                return pts

            def mk_ctx_chunks(j, h, pts):
                # ctx matmuls for head h as small closures, ascending pair
                # order (pair 0's chunk 0 covers the full q range so its
                # start=True zeroes the whole psum). The psum tile is
                # allocated lazily by the first closure.
                nch = 4 * (j + 1)
                box = [None]
                out = []

                def pair_go(pp_):
                    def go():
                        if box[0] is None:
                            box[0] = cxp.tile([P, TB], f32, tag="ctx",
                                              name=f"cps_{h}_{j}")
                        for t in (0, 1):
                            c = 2 * pp_ + t
                            qlo = max(0, 128 * c - TB * j)
                            nmm = 2 * pp_ + t
                            nc.tensor.matmul(
                                box[0][0:65, qlo:TB],
                                lhsT=v[c][:, h * VSTRIDE:h * VSTRIDE + 65],
                                rhs=pts[pp_][:, t * TB + qlo:(t + 1) * TB],
                                start=(nmm == 0), stop=(nmm == nch - 1),
                                skip_group_check=True)
                    return go

                for pp_ in range(nch // 2):
                    out.append(pair_go(pp_))
                out.append(lambda: pending.append(
                    mk_norm(h, j, h // 2, (h % 2) * 64, box[0])))
                return out

            def attend(j, ilq, look):
                reserve = ilq[-2:]
                main = ilq[:max(0, len(ilq) - 2)]
                ctxq = []

                def filler(k):
                    for _ in range(k):
                        if ctxq:
                            ctxq.pop(0)()
                        elif main:
                            main.pop(0)()
                            return

                for h in range(HL):
                    pts = pts_store.pop((j, h), None)
                    if pts is None:
                        pts = s_phase(j, h, filler)
                    else:
                        filler(3)
                    while ctxq:
                        ctxq.pop(0)()
                    if pending and h >= 2:
                        pending.pop(0)()
                    ctxq = mk_ctx_chunks(j, h, pts)
                while ctxq:
                    ctxq.pop(0)()
                while main:
                    main.pop(0)()
                # lookahead: S + exp for the first heads of block j+1 so the
                # scalar engine never starves at block boundaries (block 3 is
                # exp-bound; its S work can start as soon as proj(j+1) lands)
                def rfiller(k):
                    if reserve:
                        reserve.pop(0)()
                for h2 in look:
                    pts_store[(j + 1, h2)] = s_phase(j + 1, h2, rfiller)
                while reserve:
                    reserve.pop(0)()
                while pending:
                    pending.pop(0)()

            xq_t, xb_t = xq_0, xb_0
            for g in project_groups(0, xq_t, xb_t):
                g()
            LOOK = [[0, 1], [0, 1, 2], [0, 1], []]
            for tb in range(NTB):
                ilq = []
                if tb + 1 < NTB:
                    xq_t, xb_t = load_x(tb + 1)
                    ilq += project_groups(tb + 1, xq_t, xb_t)
                if tb >= 1:
                    ilq += output_groups(tb - 1)
                attend(tb, ilq, LOOK[tb])
            for g in output_groups(NTB - 1):
                g()

    nc.compile()
    return nc


def make_in_maps(x, Wq, Wk, Wv, Wo):
    import ml_dtypes

    f8 = ml_dtypes.float8_e4m3
    b16 = ml_dtypes.bfloat16
    x = np.asarray(x, np.float32)
    Wq, Wk, Wv, Wo = (np.asarray(w, np.float32) for w in (Wq, Wk, Wv, Wo))

    def dr_layout(a):
        # [C, N] -> DoubleRow fp8 [C/2, 2*N]: channel ch = 256*dc + 128*t + p
        Cd, N = a.shape
        return np.ascontiguousarray(
            a.reshape(Cd // 256, 2, 128, N).transpose(0, 2, 1, 3)
            .reshape(Cd // 2, 2 * N).astype(f8))

    tri = (np.arange(128)[None, :] >= np.arange(128)[:, None]).astype(b16)
    zero8 = np.zeros((128, T_FULL), f8)

    in_maps = []
    for core in range(NCORES):
        b, g = divmod(core, GROUPS)
        sl = slice(g * OC, (g + 1) * OC)
        xT = np.ascontiguousarray(x[b].T)
        in_maps.append({
            "xq8": dr_layout(xT),
            "xb16": xT.astype(b16),
            "wq8": dr_layout(np.ascontiguousarray(Wq[sl, :].T) * WSCALE),
            "wk8": dr_layout(np.ascontiguousarray(Wk[sl, :].T) * WSCALE),
            "wv16": np.ascontiguousarray(Wv[sl, :].T).astype(b16),
            "wo16": np.ascontiguousarray(Wo[:, sl].T).astype(b16),
            "tri": tri,
            "zero8": zero8,
        })
    return in_maps


def _run(inputs, trace=False):
    from concourse.bass_utils import run_bass_kernel_spmd

    nc = build_program()
    in_maps = make_in_maps(
        inputs["x"], inputs["Wq"], inputs["Wk"], inputs["Wv"], inputs["Wo"])
    res = run_bass_kernel_spmd(nc, in_maps, core_ids=list(range(NCORES)), trace=trace)
    y = np.zeros((B, T_FULL, C), np.float32)
    for core in range(NCORES):
        y[core // GROUPS] += res.results[core]["yT"].astype(np.float32).T
    return y, res


def kernel(**inputs):
    y, _ = _run(inputs)
    return y
